# revision 19
# baseline (speedup 1.0000x reference)
"""Cox proportional-hazards loss (CoxNNet) on 8 Trainium2 NeuronCores.

loss = -mean((theta - log(risk_sum)) * events)
risk_sum[i] = sum_j [d_j >= d_i] * exp(theta_j)        (N = 16384)

Sharding: rows i of the [N, N] risk-set reduction are split across 8 cores
(2048 rows each). Each core receives ONLY its shard — one packed [6144] f32
input (d_shard | theta_shard | events_shard, 24 KiB) — and reconstructs the
full d/theta vectors on-device with a single HBM-HBM AllGather over the 8
cores (16 KiB per core on NeuronLink vs. ~1 MiB of host-replicated input
over the axon tunnel at ~10 ms/MiB).

Per-core compute:
  - d, theta land in a [128, 128] chunk layout (partition = fast index),
  - w = exp(theta) on the scalar engine,
  - the core's 2048 d_i values are broadcast across 128 partitions (K=1
    matmul),
  - for each of 128 j-chunks: DVE (cols 0-1535) and GPSIMD (cols 1536-2047)
    build the [128-j x 2048-i] exact is_le 0/1 mask; 4 K=1 float32r matmuls
    (full-rate PE) accumulate risk_sum into 4 [1, 512] PSUM banks,
  - epilogue: risk -> ln -> (theta_i - ln) * e_i -> free-dim reduce
    -> one f32 partial per core.
Host combines: loss = -(sum of partials) / N.

Launch path: the stock run_bass_kernel_spmd re-creates and re-jits a fresh
shard_map closure on EVERY call (~180 ms of retrace + lowering per launch).
This module builds the jitted executable ONCE and reuses it, so a
steady-state launch is a single axon round-trip (transfers + execute +
fetch pipeline into one sync).
"""

import numpy as np

import concourse.bass as bass
import concourse.bacc as bacc
import concourse.mybir as mybir
from concourse.tile import TileContext

N = 16384
P = 128
NCH = N // P            # 128 j-chunks per core (all j)
NCORES = 8
NI = N // NCORES        # 2048 i-rows per core
FT = 512                # fp32 moving-operand max / one PSUM bank
NF = NI // FT           # 4 PSUM accumulators

# Mask generation engine split: DVE tensor_scalar(is_le) takes cols
# [0, DVE_COLS); GPSIMD takes the rest with the same exact is_le (idle
# after the prelude collective; ~0.92 ns/col at 0.6x roofline + 95 ns
# launch per instruction). ACT Sign lost its seat: ~1.4 ns/col + ~1.3 µs
# fixed beats neither, and is_le everywhere needs no tie fixup. 1536/512
# puts DVE (~108 µs) just under the PE stream (~113 µs).
DVE_COLS = 1536
F32 = mybir.dt.float32
# Main-loop matmuls stream f32 data as float32r: full-rate PE (1 cycle/row
# at moving free dim >= 512 vs 4 for plain fp32). The 0/1 masks are exact
# under any mantissa truncation; only w picks up <=1e-3 per-term error on
# hardware (fp32r is the one dtype where CoreSim and HW numerics may
# diverge), far inside the 2e-2 gate. The d-broadcast matmuls stay plain
# fp32 so bc_di is bit-exact for the is_le comparisons.
F32R = mybir.dt.float32r


def _build(dve_cols: int = DVE_COLS):
    gp_cols = NI - dve_cols
    # disable_frame_to_traceback keeps python file/line out of the BIR, so
    # the compiled artifact (and its cache key) is independent of the
    # directory kernel.py is imported from — a fresh checkout reuses the
    # warmed compile instead of paying the ~2 min neuronx-cc run.
    nc = bacc.Bacc(num_devices=NCORES, disable_frame_to_traceback=True)
    packed = nc.declare_dram_parameter("packed", [3 * NI], F32, isOutput=False)
    out = nc.declare_dram_parameter("partial", [1, 1], F32, isOutput=True)

    with TileContext(nc) as tc:
        with (
            tc.tile_pool(name="dram", bufs=1, space="DRAM") as dpool,
            tc.tile_pool(name="const", bufs=1) as cpool,
            tc.tile_pool(name="mask", bufs=4) as mpool,
            tc.tile_pool(name="acc", bufs=1, space="PSUM") as ppool,
            tc.tile_pool(name="bc", bufs=2, space="PSUM") as bcpool,
        ):
            # collective bounce buffers (collectives can't touch I/O tensors)
            dt_in = dpool.tile([1, 2 * NI], F32)     # my d_shard | theta_shard
            dt_all = dpool.tile([1, 2 * N], F32)     # 8x (d_g | theta_g)

            sb_d = cpool.tile([P, NCH], F32)    # d[p*128 + c] at [p, c]
            sb_th = cpool.tile([P, NCH], F32)
            w_act = cpool.tile([P, NCH], F32)   # exp(theta), ACT-written
            # f32r: BIR requires fp32r matmul operands to be written
            # f32r-rounded by their producer, so the tile is typed f32r and
            # the DVE copy does the rounding.
            w_sb = cpool.tile([P, NCH], F32R)   # DVE copy (single-engine deps for PE)
            ones_row = cpool.tile([1, P], F32)  # bcast lhsT [K=1, M=128]
            row_di = cpool.tile([1, NI], F32)
            row_di2 = cpool.tile([1, NI], F32)  # DVE copy of row_di
            row_thi = cpool.tile([1, NI], F32)
            row_ei = cpool.tile([1, NI], F32)
            bc_di = cpool.tile([P, NI], F32)
            risk_row = cpool.tile([1, NI], F32)
            ln_row = cpool.tile([1, NI], F32)
            diff_row = cpool.tile([1, NI], F32)
            prod_row = cpool.tile([1, NI], F32)
            part_sb = cpool.tile([1, 1], F32)

            # ---- gather the full d/theta from the 8 shards ----
            nc.gpsimd.dma_start(
                out=dt_in[:, :],
                in_=packed[:2 * NI].rearrange("(o n) -> o n", o=1),
            )
            nc.gpsimd.collective_compute(
                "AllGather",
                mybir.AluOpType.bypass,
                replica_groups=[list(range(NCORES))],
                ins=[dt_in.opt()],
                outs=[dt_all.opt()],
            )

            # ---- loads ----
            # dt_all[g*4096 : g*4096+2048] = d for global rows [2048g, 2048(g+1))
            # → sb_d[16g:16(g+1), :] (global j = p*128 + c sits at [p, c]);
            # next 2048 are the matching theta block. Per-g DMAs are forced:
            # the interleaved d|theta gather layout makes the (g, p') partition
            # strides non-mergeable (4096 vs 16*128) into one uniform-stride AP.
            for g in range(NCORES):
                o = g * 2 * NI
                nc.sync.dma_start(
                    out=sb_d[16 * g:16 * (g + 1), :],
                    in_=dt_all[0, o:o + NI].rearrange("(p c) -> p c", p=16),
                )
                nc.sync.dma_start(
                    out=sb_th[16 * g:16 * (g + 1), :],
                    in_=dt_all[0, o + NI:o + 2 * NI].rearrange("(p c) -> p c", p=16),
                )
            nc.sync.dma_start(out=row_di[:, :],
                              in_=packed[:NI].rearrange("(o n) -> o n", o=1))
            nc.sync.dma_start(out=row_thi[:, :],
                              in_=packed[NI:2 * NI].rearrange("(o n) -> o n", o=1))
            nc.sync.dma_start(out=row_ei[:, :],
                              in_=packed[2 * NI:].rearrange("(o n) -> o n", o=1))

            # ---- prep ----
            # PE allows only ONE sync wait per Matmult: funnel every matmul
            # input through the vector engine so PE waits on a single DVE sem.
            nc.scalar.activation(w_act[:, :], sb_th[:, :], mybir.ActivationFunctionType.Exp)
            nc.vector.tensor_copy(w_sb[:, :], w_act[:, :])
            nc.vector.memset(ones_row[:, :], 1.0)
            nc.vector.tensor_copy(row_di2[:, :], row_di[:, :])
            for t in range(NF):
                bc_ps = bcpool.tile([P, FT], F32, tag="bc")
                nc.tensor.matmul(
                    bc_ps[:, :], lhsT=ones_row[:, :],
                    rhs=row_di2[:, t * FT:(t + 1) * FT], start=True, stop=True,
                )
                nc.vector.tensor_copy(bc_di[:, t * FT:(t + 1) * FT], bc_ps[:, :])

            # ---- main loop: mask gen + masked reduce ----
            risk_ps = [ppool.tile([1, FT], F32, name=f"risk{t}") for t in range(NF)]
            assert dve_cols % FT == 0, "engine split must align to matmul tiles"
            for c in range(NCH):
                # separate tiles per producing engine — a shared tile would
                # WAW-serialize DVE behind GPSIMD in the Tile dep tracker
                mask_d = None
                mask_g = None
                if dve_cols > 0:
                    mask_d = mpool.tile([P, dve_cols], F32R, tag="mask_d",
                                        name=f"mask_d{c}")
                if gp_cols > 0:
                    mask_g = mpool.tile([P, gp_cols], F32R, tag="mask_g",
                                        name=f"mask_g{c}")
                if mask_d is not None:
                    nc.vector.tensor_scalar(
                        mask_d[:, :], bc_di[:, :dve_cols],
                        sb_d[:, c:c + 1], None, mybir.AluOpType.is_le,
                    )
                if mask_g is not None:
                    nc.gpsimd.tensor_scalar(
                        mask_g[:, :], bc_di[:, dve_cols:],
                        sb_d[:, c:c + 1], None, mybir.AluOpType.is_le,
                    )
                for t in range(NF):
                    lo = t * FT
                    if lo < dve_cols:
                        rhs = mask_d[:, lo:lo + FT]
                    else:
                        rhs = mask_g[:, lo - dve_cols:lo - dve_cols + FT]
                    nc.tensor.matmul(
                        risk_ps[t][:, :], lhsT=w_sb[:, c:c + 1],
                        rhs=rhs,
                        start=(c == 0), stop=(c == NCH - 1),
                    )

            # ---- epilogue ----
            for t in range(NF):
                nc.vector.tensor_copy(risk_row[:, t * FT:(t + 1) * FT],
                                      risk_ps[t][:, :])

            # (tensor_tensor_reduce crashes at runtime on this stack — use
            # separate mul + reduce_sum instead)
            nc.scalar.activation(ln_row[:, :], risk_row[:, :],
                                 mybir.ActivationFunctionType.Ln)
            nc.vector.tensor_sub(diff_row[:, :], row_thi[:, :], ln_row[:, :])
            nc.vector.tensor_mul(prod_row[:, :], diff_row[:, :], row_ei[:, :])
            nc.vector.reduce_sum(part_sb[:, :], prod_row[:, :],
                                 axis=mybir.AxisListType.X)
            nc.sync.dma_start(out=out[:, :], in_=part_sb[:, :])

    nc.finalize()
    return nc


def _make_cached_runner(nc):
    """One-time: lower nc to a jitted shard_map executable and keep it.

    Mirrors bass2jax.run_bass_via_pjrt, but hoists everything reusable out
    of the per-call path — the stock helper rebuilds + re-jits a fresh
    closure per call, which costs ~180 ms of retrace/lowering per launch.
    """
    import jax
    from jax.experimental.shard_map import shard_map
    from jax.sharding import Mesh, PartitionSpec

    from concourse.bass2jax import (
        _bass_exec_p,
        install_neuronx_cc_hook,
        partition_id_tensor,
    )

    install_neuronx_cc_hook()

    # The serialized BIR is embedded verbatim in the HLO custom_call, so
    # every compile-cache key downstream inherits its byte content. The
    # ant_debug strings embed this file's absolute path, which would make
    # the cache key depend on the directory kernel.py is imported from
    # (fresh checkout -> ~2 min recompile). Scrub the path so the artifact
    # is byte-identical everywhere; debug info is otherwise untouched.
    import os
    _path = os.path.abspath(__file__).encode()
    _orig_to_json_bytes = nc.to_json_bytes

    def _scrubbed_to_json_bytes():
        return _orig_to_json_bytes().replace(_path, b"kernel.py")

    nc.to_json_bytes = _scrubbed_to_json_bytes

    partition_name = nc.partition_id_tensor.name if nc.partition_id_tensor else None

    in_names, out_names, out_avals, out_shapes = [], [], [], []
    for alloc in nc.m.functions[0].allocations:
        if not isinstance(alloc, mybir.MemoryLocationSet):
            continue
        name = alloc.memorylocations[0].name
        if alloc.kind == "ExternalInput":
            if name != partition_name:
                in_names.append(name)
        elif alloc.kind == "ExternalOutput":
            out_names.append(name)
            shape = tuple(alloc.tensor_shape)
            out_shapes.append(shape)
            out_avals.append(jax.core.ShapedArray(shape, mybir.dt.np(alloc.dtype)))
    assert in_names == ["packed"] and out_names == ["partial"]
    n_params = len(in_names)
    n_outs = len(out_avals)
    all_in_names = in_names + out_names
    if partition_name is not None:
        all_in_names.append(partition_name)
    donate = tuple(range(n_params, n_params + n_outs))

    def _body(*args):
        operands = list(args)
        if partition_name is not None:
            operands.append(partition_id_tensor())
        outs = _bass_exec_p.bind(
            *operands,
            out_avals=tuple(out_avals),
            in_names=tuple(all_in_names),
            out_names=tuple(out_names),
            lowering_input_output_aliases=(),
            sim_require_finite=True,
            sim_require_nnan=True,
            nc=nc,
        )
        return tuple(outs)

    devices = jax.devices()[:NCORES]
    assert len(devices) == NCORES, f"need {NCORES} cores, have {len(jax.devices())}"
    mesh = Mesh(np.asarray(devices), ("core",))
    in_specs = (PartitionSpec("core"),) * (n_params + n_outs)
    out_specs = (PartitionSpec("core"),) * n_outs
    sharded = jax.jit(
        shard_map(_body, mesh=mesh, in_specs=in_specs, out_specs=out_specs,
                  check_rep=False),
        donate_argnums=donate,
        keep_unused=True,
    )

    def run(packed_all):
        """packed_all: [NCORES * 3*NI] f32 → per-core partial sums [NCORES]."""
        zeros = np.zeros((NCORES * out_shapes[0][0], *out_shapes[0][1:]),
                         np.float32)
        out_arrs = sharded(packed_all, zeros)
        return np.asarray(out_arrs[0]).reshape(-1)

    return run


_RUNNER = None


def _get_runner():
    global _RUNNER
    if _RUNNER is None:
        _RUNNER = _make_cached_runner(_build())
    return _RUNNER


def _pack_inputs(hazard_pred, durations, events):
    theta = np.asarray(hazard_pred, dtype=np.float32).reshape(-1)
    d = np.asarray(durations, dtype=np.float32).reshape(-1)
    e = np.asarray(events, dtype=np.float32).reshape(-1)
    packed = np.empty((NCORES, 3 * NI), np.float32)
    packed[:, :NI] = d.reshape(NCORES, NI)
    packed[:, NI:2 * NI] = theta.reshape(NCORES, NI)
    packed[:, 2 * NI:] = e.reshape(NCORES, NI)
    return packed.reshape(-1)


def kernel(hazard_pred, durations, events):
    runner = _get_runner()
    partials = runner(_pack_inputs(hazard_pred, durations, events))
    loss = -(np.sum(partials.astype(np.float64)) / N)
    return np.asarray(loss, dtype=np.float32)


def run(hazard_pred, durations, events, trace=False, dve_cols=DVE_COLS, **kw):
    """test.py compatibility shim (trace/dve_cols accepted and ignored)."""
    return kernel(hazard_pred, durations, events), None
